# revision 13
# baseline (speedup 1.0000x reference)
"""Local (banded) sparse attention on 8 Trainium2 NeuronCores.

Problem: x [4, 4096, 512] f32; Q/K/V = x@W + b; scores masked to |i-j| <= 128
(window 257); softmax; out = attn @ V. Output [4, 4096, 512] f32.

Sharding: 8 cores = 4 batches x 2 sequence halves. Each core computes 2048
query rows using a 2304-row context (its half plus a 128-token halo on each
side, zero-padded at the global sequence edges; edge positions are excluded
by the additive mask so the padding value never matters).

Layout strategy (per core):
  xt  [512, 2304]  x-context transposed, d on partitions (4 chunks of 128)
  qT  [512, 2048]  Q^T = Wq^T-free matmul: lhsT=Wq chunk, rhs=xt  (d' on part)
  kT  [512, 2304]  same with Wk
  v   18 x [128, 512] V tiles, sequence on partitions: lhsT=xt chunk, rhs=Wv
  per 128-query block: scores[128,384] = qT_chunk^T@kT stripe; additive band
  mask; softmax along free dim (ACT exp w/ accum_out sum); P transposed via
  PE; out = P^T-chunks^T @ v tiles; row-scale by 1/sum.
"""

import math

import numpy as np

import concourse.bass as bass
import concourse.mybir as mybir
import concourse.tile as tile
from concourse import bacc
from concourse.bass_utils import run_bass_kernel_spmd

B, S, D = 4, 4096, 512
HALF_W = 128  # half window; window size = 257
N_CORES = 8
SQ = S // 2  # 2048 query rows per core
SCTX = SQ + 2 * HALF_W  # 2304 context rows per core
NQB = SQ // 128  # 16 query blocks
NKT = SCTX // 128  # 18 context tiles
DC = D // 128  # 4 contraction chunks
STRIPE = 3 * 128  # 384 key stripe per query block
SCALE = 1.0 / math.sqrt(D)
NEG = -1e30

# matmul input dtype: float32 (exact, 4 cyc/row) or float32r (TF32-ish, 1 cyc/row
# for free dim >= 256). walrus requires fp32r matmul inputs to be *produced* as
# fp32r, so the feeding DRAM tensors / SBUF tiles carry the dtype natively
# (same 4-byte layout; host arrays stay np.float32).
MM_DT = mybir.dt.float32r

_CACHE = {}


def build_program(use_bias=False):
    key = ("nc", use_bias)
    if key in _CACHE:
        return _CACHE[key]
    f32 = mybir.dt.float32
    nc = bacc.Bacc("TRN2", target_bir_lowering=False, debug=False)

    xt = nc.dram_tensor("xt", [D, SCTX], MM_DT, kind="ExternalInput").ap()
    wq = nc.dram_tensor("wq", [D, D], MM_DT, kind="ExternalInput").ap()
    wk = nc.dram_tensor("wk", [D, D], MM_DT, kind="ExternalInput").ap()
    wv = nc.dram_tensor("wv", [D, D], MM_DT, kind="ExternalInput").ap()
    bq = nc.dram_tensor("bq", [D], f32, kind="ExternalInput").ap()
    bk = nc.dram_tensor("bk", [D], f32, kind="ExternalInput").ap()
    bv = nc.dram_tensor("bv", [D], f32, kind="ExternalInput").ap()
    masks = nc.dram_tensor("masks", [3, 128, STRIPE], MM_DT, kind="ExternalInput").ap()
    ident_d = nc.dram_tensor("ident", [128, 128], MM_DT, kind="ExternalInput").ap()
    out = nc.dram_tensor("out", [SQ, D], f32, kind="ExternalOutput").ap()

    with tile.TileContext(nc) as tc:
        with (
            tc.tile_pool(name="consts", bufs=1) as consts,
            tc.tile_pool(name="persist", bufs=1) as persist,
            tc.tile_pool(name="vpool", bufs=8) as vpool,
            tc.tile_pool(name="work", bufs=3) as work,
            tc.tile_pool(name="stats", bufs=12) as stats,
            tc.tile_pool(name="outp", bufs=3) as outp,
            tc.tile_pool(name="ps_proj", bufs=2, space="PSUM") as ps_proj,
            tc.tile_pool(name="ps_sc", bufs=2, space="PSUM") as ps_sc,
            tc.tile_pool(name="ps_pt", bufs=2, space="PSUM") as ps_pt,
            tc.tile_pool(name="ps_av", bufs=2, space="PSUM") as ps_av,
        ):
            # ---- constants into SBUF ----
            # DMA order is tuned so compute can chase the stream: wk first,
            # then xt piecewise in s-major [128,512] blocks (each s-chunk
            # delivers all 4 contraction chunks for that s range), weights for
            # q/v early, small constants last.
            s_chunks = [(i * 512, min(512, SCTX - i * 512)) for i in range((SCTX + 511) // 512)]
            xt_s = consts.tile([128, DC, SCTX], MM_DT, tag="xt")
            xt_r = xt.rearrange("(c p) s -> p c s", p=128)
            w_s = {}

            def load_w(name, w):
                t = consts.tile([128, DC, D], MM_DT, tag=name)
                nc.gpsimd.dma_start(out=t, in_=w.rearrange("(c p) n -> p c n", p=128))
                w_s[name] = t

            load_w("wk", wk)
            for k in range(DC):
                nc.sync.dma_start(
                    out=xt_s[:, k, 0:512], in_=xt_r[:, k, 0:512]
                )
            load_w("wq", wq)
            load_w("wv", wv)
            for s0, slen in s_chunks[1:]:
                for k in range(DC):
                    nc.sync.dma_start(
                        out=xt_s[:, k, s0 : s0 + slen], in_=xt_r[:, k, s0 : s0 + slen]
                    )
            masks_s = consts.tile([128, 3, STRIPE], MM_DT, tag="masks")
            nc.gpsimd.dma_start(out=masks_s, in_=masks.rearrange("m p c -> p m c"))
            ident = consts.tile([128, 128], MM_DT, tag="ident")
            nc.gpsimd.dma_start(out=ident, in_=ident_d)
            b_s = {}
            bv_s = None
            if use_bias:
                for name, b in (("bq", bq), ("bk", bk)):
                    t = consts.tile([128, DC], f32, tag=name)
                    nc.sync.dma_start(out=t, in_=b.rearrange("(c p) -> p c", p=128))
                    b_s[name] = t
                bv_s = consts.tile([128, D], f32, tag="bv")
                nc.sync.dma_start(
                    out=bv_s,
                    in_=bass.AP(tensor=bv.tensor, offset=bv.offset, ap=[[0, 128]] + bv.ap),
                )

            # ---- phase 1: projections, s-outer so compute chases the DMA ----
            kT_s = persist.tile([128, DC, SCTX], MM_DT, tag="kT")
            qT_s = persist.tile([128, DC, SQ], MM_DT, tag="qT")
            q_chunks = [(i * 512, 512) for i in range(SQ // 512)]
            v_tiles = [None] * NKT

            def proj_k(m, s0, slen):
                ps = ps_proj.tile([128, 512], f32, tag="proj")
                for k in range(DC):
                    nc.tensor.matmul(
                        ps[:, :slen],
                        lhsT=w_s["wk"][:, k, m * 128 : (m + 1) * 128],
                        rhs=xt_s[:, k, s0 : s0 + slen],
                        start=(k == 0),
                        stop=(k == DC - 1),
                    )
                if use_bias:
                    nc.vector.tensor_scalar_add(
                        kT_s[:, m, s0 : s0 + slen], ps[:, :slen], b_s["bk"][:, m : m + 1]
                    )
                else:
                    nc.vector.tensor_copy(kT_s[:, m, s0 : s0 + slen], ps[:, :slen])

            def proj_q(m, s0, slen):
                ps = ps_proj.tile([128, 512], f32, tag="proj")
                for k in range(DC):
                    nc.tensor.matmul(
                        ps[:, :slen],
                        lhsT=w_s["wq"][:, k, m * 128 : (m + 1) * 128],
                        rhs=xt_s[:, k, HALF_W + s0 : HALF_W + s0 + slen],
                        start=(k == 0),
                        stop=(k == DC - 1),
                    )
                if use_bias:
                    nc.vector.tensor_scalar_add(
                        qT_s[:, m, s0 : s0 + slen], ps[:, :slen], b_s["bq"][:, m : m + 1]
                    )
                else:
                    nc.vector.tensor_copy(qT_s[:, m, s0 : s0 + slen], ps[:, :slen])

            def proj_v(t):
                ps = ps_proj.tile([128, 512], f32, tag="proj")
                for k in range(DC):
                    nc.tensor.matmul(
                        ps,
                        lhsT=xt_s[:, k, t * 128 : (t + 1) * 128],
                        rhs=w_s["wv"][:, k, :],
                        start=(k == 0),
                        stop=(k == DC - 1),
                    )
                vt = vpool.tile([128, D], MM_DT, tag="v")
                if use_bias:
                    nc.vector.tensor_add(vt, ps, bv_s)
                else:
                    nc.vector.tensor_copy(vt, ps)
                v_tiles[t] = vt

            for si, (s0, slen) in enumerate(s_chunks):
                for m in range(DC):
                    proj_k(m, s0, slen)
                for t in range(s0 // 128, (s0 + slen) // 128):
                    proj_v(t)
                # qT chunk si-1 needs xt rows [128+s0-512, 128+s0), available
                # once chunk si has landed
                if si >= 1 and si - 1 < len(q_chunks):
                    q0, qlen = q_chunks[si - 1]
                    for m in range(DC):
                        proj_q(m, q0, qlen)

            # ---- phase 2: banded attention per 128-query block ----
            # scores PSUM group = 4 QK matmuls + 1 identity@mask matmul (adds
            # the additive band mask on the PE, no DVE pass needed). No
            # max-subtraction: scaled scores are bounded (|s| <= |q||k|/sqrt(D)
            # ~ 23) so exp cannot overflow in fp32; softmax is shift-invariant.
            for qb in range(NQB):
                mi = 0 if qb == 0 else (2 if qb == NQB - 1 else 1)
                ps_scores = ps_sc.tile([128, STRIPE], f32, tag="sc")
                for k in range(DC):
                    nc.tensor.matmul(
                        ps_scores,
                        lhsT=qT_s[:, k, qb * 128 : (qb + 1) * 128],
                        rhs=kT_s[:, k, qb * 128 : qb * 128 + STRIPE],
                        start=(k == 0),
                        stop=False,
                    )
                nc.tensor.matmul(
                    ps_scores,
                    lhsT=ident,
                    rhs=masks_s[:, mi, :],
                    start=False,
                    stop=True,
                )
                p_t = work.tile([128, STRIPE], MM_DT, tag="p")
                ssum = stats.tile([128, 1], f32, tag="ss")
                nc.scalar.activation(
                    out=p_t,
                    in_=ps_scores,
                    func=mybir.ActivationFunctionType.Exp,
                    bias=0.0,
                    scale=SCALE,
                    accum_out=ssum,
                )
                rinv = stats.tile([128, 1], f32, tag="ri")
                nc.vector.reciprocal(rinv, ssum)
                pt_psum = ps_pt.tile([128, STRIPE], MM_DT, tag="pt")
                for j in range(3):
                    nc.tensor.transpose(
                        pt_psum[:, j * 128 : (j + 1) * 128],
                        p_t[:, j * 128 : (j + 1) * 128],
                        ident,
                    )
                pt_s = work.tile([128, STRIPE], MM_DT, tag="pts")
                nc.vector.tensor_copy(pt_s, pt_psum)
                av = ps_av.tile([128, D], f32, tag="av")
                for j in range(3):
                    nc.tensor.matmul(
                        av,
                        lhsT=pt_s[:, j * 128 : (j + 1) * 128],
                        rhs=v_tiles[qb + j],
                        start=(j == 0),
                        stop=(j == 2),
                    )
                o_t = outp.tile([128, D], f32, tag="o")
                nc.scalar.activation(
                    out=o_t,
                    in_=av,
                    func=mybir.ActivationFunctionType.Copy,
                    bias=0.0,
                    scale=rinv,
                )
                nc.sync.dma_start(out=out[qb * 128 : (qb + 1) * 128, :], in_=o_t)

    nc.compile()
    _CACHE[key] = nc
    return nc


def _band_masks(h):
    """Additive masks [3, 128, 384] for (first, interior, last) query blocks of
    this core's half h (0=left half of sequence, 1=right)."""
    r = np.arange(128)[:, None]
    c = np.arange(STRIPE)[None, :]
    base = np.where((c >= r) & (c <= r + 2 * HALF_W), 0.0, NEG).astype(np.float32)
    first = base + np.where(c >= HALF_W, 0.0, NEG).astype(np.float32)
    last = base + np.where(c < STRIPE - HALF_W, 0.0, NEG).astype(np.float32)
    m = np.stack([base, base, base])
    if h == 0:
        m[0] = first
    else:
        m[2] = last
    return m


def make_in_maps(x, Wq, bq, Wk, bk, Wv, bv):
    x = np.asarray(x, dtype=np.float32)
    Wq, Wk, Wv = (np.ascontiguousarray(np.asarray(a, np.float32)) for a in (Wq, Wk, Wv))
    bq, bk, bv = (np.ascontiguousarray(np.asarray(a, np.float32)) for a in (bq, bk, bv))
    in_maps = []
    for core in range(N_CORES):
        b, h = divmod(core, 2)
        lo, hi = h * SQ - HALF_W, h * SQ + SQ + HALF_W
        ctx = np.zeros((SCTX, D), np.float32)
        s0, s1 = max(lo, 0), min(hi, S)
        ctx[s0 - lo : s1 - lo] = x[b, s0:s1]
        in_maps.append(
            {
                "xt": np.ascontiguousarray(ctx.T),
                "wq": Wq, "wk": Wk, "wv": Wv,
                "bq": bq, "bk": bk, "bv": bv,
                "masks": _band_masks(h),
                "ident": np.eye(128, dtype=np.float32),
            }
        )
    return in_maps


def kernel(x, Wq, bq, Wk, bk, Wv, bv, **run_kwargs):
    use_bias = any(np.any(np.asarray(b)) for b in (bq, bk, bv))
    nc = build_program(use_bias=use_bias)
    in_maps = make_in_maps(x, Wq, bq, Wk, bk, Wv, bv)
    res = run_bass_kernel_spmd(nc, in_maps, core_ids=list(range(N_CORES)), **run_kwargs)
    out = np.empty((B, S, D), np.float32)
    for core in range(N_CORES):
        b, h = divmod(core, 2)
        out[b, h * SQ : (h + 1) * SQ] = res.results[core]["out"]
    if run_kwargs:
        kernel.last_result = res
    return out


# revision 14
# speedup vs baseline: 1.0836x; 1.0836x over previous
"""Local (banded) sparse attention on 8 Trainium2 NeuronCores.

Problem: x [4, 4096, 512] f32; Q/K/V = x@W + b; scores masked to |i-j| <= 128
(window 257); softmax; out = attn @ V. Output [4, 4096, 512] f32.

Sharding: 8 cores = 4 batches x 2 sequence halves. Each core computes 2048
query rows using a 2304-row context (its half plus a 128-token halo on each
side, zero-padded at the global sequence edges; edge positions are excluded
by the additive mask so the padding value never matters).

Layout strategy (per core):
  xt  [512, 2304]  x-context transposed, d on partitions (4 chunks of 128)
  qT  [512, 2048]  Q^T = Wq^T-free matmul: lhsT=Wq chunk, rhs=xt  (d' on part)
  kT  [512, 2304]  same with Wk
  v   18 x [128, 512] V tiles, sequence on partitions: lhsT=xt chunk, rhs=Wv
  per 128-query block: scores[128,384] = qT_chunk^T@kT stripe; additive band
  mask; softmax along free dim (ACT exp w/ accum_out sum); P transposed via
  PE; out = P^T-chunks^T @ v tiles; row-scale by 1/sum.
"""

import math

import numpy as np

import concourse.bass as bass
import concourse.mybir as mybir
import concourse.tile as tile
from concourse import bacc
from concourse.bass_utils import run_bass_kernel_spmd

B, S, D = 4, 4096, 512
HALF_W = 128  # half window; window size = 257
N_CORES = 8
SQ = S // 2  # 2048 query rows per core
SCTX = SQ + 2 * HALF_W  # 2304 context rows per core
NQB = SQ // 128  # 16 query blocks
NKT = SCTX // 128  # 18 context tiles
DC = D // 128  # 4 contraction chunks
STRIPE = 3 * 128  # 384 key stripe per query block
SCALE = 1.0 / math.sqrt(D)
NEG = -1e30

# matmul input dtype: float32 (exact, 4 cyc/row) or float32r (TF32-ish, 1 cyc/row
# for free dim >= 256). walrus requires fp32r matmul inputs to be *produced* as
# fp32r, so the feeding DRAM tensors / SBUF tiles carry the dtype natively
# (same 4-byte layout; host arrays stay np.float32).
MM_DT = mybir.dt.float32r

_CACHE = {}


def build_program(use_bias=False):
    key = ("nc", use_bias)
    if key in _CACHE:
        return _CACHE[key]
    f32 = mybir.dt.float32
    nc = bacc.Bacc("TRN2", target_bir_lowering=False, debug=False)

    xt = nc.dram_tensor("xt", [D, SCTX], MM_DT, kind="ExternalInput").ap()
    wq = nc.dram_tensor("wq", [D, D], MM_DT, kind="ExternalInput").ap()
    wk = nc.dram_tensor("wk", [D, D], MM_DT, kind="ExternalInput").ap()
    wv = nc.dram_tensor("wv", [D, D], MM_DT, kind="ExternalInput").ap()
    bq = nc.dram_tensor("bq", [D], f32, kind="ExternalInput").ap()
    bk = nc.dram_tensor("bk", [D], f32, kind="ExternalInput").ap()
    bv = nc.dram_tensor("bv", [D], f32, kind="ExternalInput").ap()
    masks = nc.dram_tensor("masks", [3, 128, STRIPE], MM_DT, kind="ExternalInput").ap()
    ident_d = nc.dram_tensor("ident", [128, 128], MM_DT, kind="ExternalInput").ap()
    out = nc.dram_tensor("out", [SQ, D], f32, kind="ExternalOutput").ap()

    with tile.TileContext(nc) as tc:
        with (
            tc.tile_pool(name="consts", bufs=1) as consts,
            tc.tile_pool(name="persist", bufs=1) as persist,
            tc.tile_pool(name="vpool", bufs=8) as vpool,
            tc.tile_pool(name="work", bufs=3) as work,
            tc.tile_pool(name="stats", bufs=12) as stats,
            tc.tile_pool(name="outp", bufs=3) as outp,
            tc.tile_pool(name="ps_proj", bufs=2, space="PSUM") as ps_proj,
            tc.tile_pool(name="ps_sc", bufs=2, space="PSUM") as ps_sc,
            tc.tile_pool(name="ps_pt", bufs=2, space="PSUM") as ps_pt,
            tc.tile_pool(name="ps_av", bufs=2, space="PSUM") as ps_av,
        ):
            # ---- constants into SBUF ----
            # DMA order is tuned so compute can chase the stream: wk first,
            # then xt piecewise in s-major [128,512] blocks (each s-chunk
            # delivers all 4 contraction chunks for that s range), weights for
            # q/v early, small constants last.
            s_chunks = [(i * 512, min(512, SCTX - i * 512)) for i in range((SCTX + 511) // 512)]
            xt_s = consts.tile([128, DC, SCTX], MM_DT, tag="xt")
            xt_r = xt.rearrange("(c p) s -> p c s", p=128)
            w_s = {}

            def load_w(name, w):
                t = consts.tile([128, DC, D], MM_DT, tag=name)
                nc.sync.dma_start(out=t, in_=w.rearrange("(c p) n -> p c n", p=128))
                w_s[name] = t

            def load_xt(si):
                s0, slen = s_chunks[si]
                nc.sync.dma_start(
                    out=xt_s[:, :, s0 : s0 + slen], in_=xt_r[:, :, s0 : s0 + slen]
                )

            load_w("wk", wk)
            load_xt(0)
            load_w("wq", wq)
            load_w("wv", wv)
            load_xt(1)
            masks_s = consts.tile([128, 3, STRIPE], MM_DT, tag="masks")
            nc.sync.dma_start(out=masks_s, in_=masks.rearrange("m p c -> p m c"))
            ident = consts.tile([128, 128], MM_DT, tag="ident")
            nc.sync.dma_start(out=ident, in_=ident_d)
            for si in range(2, len(s_chunks)):
                load_xt(si)
            b_s = {}
            bv_s = None
            if use_bias:
                for name, b in (("bq", bq), ("bk", bk)):
                    t = consts.tile([128, DC], f32, tag=name)
                    nc.sync.dma_start(out=t, in_=b.rearrange("(c p) -> p c", p=128))
                    b_s[name] = t
                bv_s = consts.tile([128, D], f32, tag="bv")
                nc.sync.dma_start(
                    out=bv_s,
                    in_=bass.AP(tensor=bv.tensor, offset=bv.offset, ap=[[0, 128]] + bv.ap),
                )

            # ---- phase 1: projections, s-outer so compute chases the DMA ----
            kT_s = persist.tile([128, DC, SCTX], MM_DT, tag="kT")
            qT_s = persist.tile([128, DC, SQ], MM_DT, tag="qT")
            q_chunks = [(i * 512, 512) for i in range(SQ // 512)]
            v_tiles = [None] * NKT

            def proj_k(m, s0, slen):
                ps = ps_proj.tile([128, 512], f32, tag="proj")
                for k in range(DC):
                    nc.tensor.matmul(
                        ps[:, :slen],
                        lhsT=w_s["wk"][:, k, m * 128 : (m + 1) * 128],
                        rhs=xt_s[:, k, s0 : s0 + slen],
                        start=(k == 0),
                        stop=(k == DC - 1),
                    )
                if use_bias:
                    nc.vector.tensor_scalar_add(
                        kT_s[:, m, s0 : s0 + slen], ps[:, :slen], b_s["bk"][:, m : m + 1]
                    )
                else:
                    nc.vector.tensor_copy(kT_s[:, m, s0 : s0 + slen], ps[:, :slen])

            def proj_q(m, s0, slen):
                ps = ps_proj.tile([128, 512], f32, tag="proj")
                for k in range(DC):
                    nc.tensor.matmul(
                        ps[:, :slen],
                        lhsT=w_s["wq"][:, k, m * 128 : (m + 1) * 128],
                        rhs=xt_s[:, k, HALF_W + s0 : HALF_W + s0 + slen],
                        start=(k == 0),
                        stop=(k == DC - 1),
                    )
                if use_bias:
                    nc.vector.tensor_scalar_add(
                        qT_s[:, m, s0 : s0 + slen], ps[:, :slen], b_s["bq"][:, m : m + 1]
                    )
                else:
                    nc.vector.tensor_copy(qT_s[:, m, s0 : s0 + slen], ps[:, :slen])

            def proj_v(t):
                ps = ps_proj.tile([128, 512], f32, tag="proj")
                for k in range(DC):
                    nc.tensor.matmul(
                        ps,
                        lhsT=xt_s[:, k, t * 128 : (t + 1) * 128],
                        rhs=w_s["wv"][:, k, :],
                        start=(k == 0),
                        stop=(k == DC - 1),
                    )
                vt = vpool.tile([128, D], MM_DT, tag="v")
                if use_bias:
                    nc.vector.tensor_add(vt, ps, bv_s)
                else:
                    nc.vector.tensor_copy(vt, ps)
                v_tiles[t] = vt

            for si, (s0, slen) in enumerate(s_chunks):
                for m in range(DC):
                    proj_k(m, s0, slen)
                for t in range(s0 // 128, (s0 + slen) // 128):
                    proj_v(t)
                # qT chunk si-1 needs xt rows [128+s0-512, 128+s0), available
                # once chunk si has landed
                if si >= 1 and si - 1 < len(q_chunks):
                    q0, qlen = q_chunks[si - 1]
                    for m in range(DC):
                        proj_q(m, q0, qlen)

            # ---- phase 2: banded attention per 128-query block ----
            # scores PSUM group = 4 QK matmuls + 1 identity@mask matmul (adds
            # the additive band mask on the PE, no DVE pass needed). No
            # max-subtraction: scaled scores are bounded (|s| <= |q||k|/sqrt(D)
            # ~ 23) so exp cannot overflow in fp32; softmax is shift-invariant.
            for qb in range(NQB):
                mi = 0 if qb == 0 else (2 if qb == NQB - 1 else 1)
                ps_scores = ps_sc.tile([128, STRIPE], f32, tag="sc")
                for k in range(DC):
                    nc.tensor.matmul(
                        ps_scores,
                        lhsT=qT_s[:, k, qb * 128 : (qb + 1) * 128],
                        rhs=kT_s[:, k, qb * 128 : qb * 128 + STRIPE],
                        start=(k == 0),
                        stop=False,
                    )
                nc.tensor.matmul(
                    ps_scores,
                    lhsT=ident,
                    rhs=masks_s[:, mi, :],
                    start=False,
                    stop=True,
                )
                p_t = work.tile([128, STRIPE], MM_DT, tag="p")
                ssum = stats.tile([128, 1], f32, tag="ss")
                nc.scalar.activation(
                    out=p_t,
                    in_=ps_scores,
                    func=mybir.ActivationFunctionType.Exp,
                    bias=0.0,
                    scale=SCALE,
                    accum_out=ssum,
                )
                rinv = stats.tile([128, 1], f32, tag="ri")
                nc.vector.reciprocal(rinv, ssum)
                pt_psum = ps_pt.tile([128, STRIPE], MM_DT, tag="pt")
                for j in range(3):
                    nc.tensor.transpose(
                        pt_psum[:, j * 128 : (j + 1) * 128],
                        p_t[:, j * 128 : (j + 1) * 128],
                        ident,
                    )
                pt_s = work.tile([128, STRIPE], MM_DT, tag="pts")
                nc.vector.tensor_copy(pt_s, pt_psum)
                av = ps_av.tile([128, D], f32, tag="av")
                for j in range(3):
                    nc.tensor.matmul(
                        av,
                        lhsT=pt_s[:, j * 128 : (j + 1) * 128],
                        rhs=v_tiles[qb + j],
                        start=(j == 0),
                        stop=(j == 2),
                    )
                o_t = outp.tile([128, D], f32, tag="o")
                nc.scalar.activation(
                    out=o_t,
                    in_=av,
                    func=mybir.ActivationFunctionType.Copy,
                    bias=0.0,
                    scale=rinv,
                )
                nc.sync.dma_start(out=out[qb * 128 : (qb + 1) * 128, :], in_=o_t)

    nc.compile()
    _CACHE[key] = nc
    return nc


def _band_masks(h):
    """Additive masks [3, 128, 384] for (first, interior, last) query blocks of
    this core's half h (0=left half of sequence, 1=right)."""
    r = np.arange(128)[:, None]
    c = np.arange(STRIPE)[None, :]
    base = np.where((c >= r) & (c <= r + 2 * HALF_W), 0.0, NEG).astype(np.float32)
    first = base + np.where(c >= HALF_W, 0.0, NEG).astype(np.float32)
    last = base + np.where(c < STRIPE - HALF_W, 0.0, NEG).astype(np.float32)
    m = np.stack([base, base, base])
    if h == 0:
        m[0] = first
    else:
        m[2] = last
    return m


def make_in_maps(x, Wq, bq, Wk, bk, Wv, bv):
    x = np.asarray(x, dtype=np.float32)
    Wq, Wk, Wv = (np.ascontiguousarray(np.asarray(a, np.float32)) for a in (Wq, Wk, Wv))
    bq, bk, bv = (np.ascontiguousarray(np.asarray(a, np.float32)) for a in (bq, bk, bv))
    in_maps = []
    for core in range(N_CORES):
        b, h = divmod(core, 2)
        lo, hi = h * SQ - HALF_W, h * SQ + SQ + HALF_W
        ctx = np.zeros((SCTX, D), np.float32)
        s0, s1 = max(lo, 0), min(hi, S)
        ctx[s0 - lo : s1 - lo] = x[b, s0:s1]
        in_maps.append(
            {
                "xt": np.ascontiguousarray(ctx.T),
                "wq": Wq, "wk": Wk, "wv": Wv,
                "bq": bq, "bk": bk, "bv": bv,
                "masks": _band_masks(h),
                "ident": np.eye(128, dtype=np.float32),
            }
        )
    return in_maps


def kernel(x, Wq, bq, Wk, bk, Wv, bv, **run_kwargs):
    use_bias = any(np.any(np.asarray(b)) for b in (bq, bk, bv))
    nc = build_program(use_bias=use_bias)
    in_maps = make_in_maps(x, Wq, bq, Wk, bk, Wv, bv)
    res = run_bass_kernel_spmd(nc, in_maps, core_ids=list(range(N_CORES)), **run_kwargs)
    out = np.empty((B, S, D), np.float32)
    for core in range(N_CORES):
        b, h = divmod(core, 2)
        out[b, h * SQ : (h + 1) * SQ] = res.results[core]["out"]
    if run_kwargs:
        kernel.last_result = res
    return out
